# revision 1
# baseline (speedup 1.0000x reference)
"""Trainium2 Bass kernel for nn_LinearPositionInterpolation.

Piecewise-linear interpolation of 65 keypoints (uniform spacing 64) up to
m=4096 output timesteps:  out[b, j, d] = (1-t_j) * v[b, seg_j, d] + t_j *
v[b, seg_j+1, d].

Strategy (per core, data-parallel over batch, 16 batches/core):
  - Express the interpolation as out[j, (b,d)] = W[j, :] @ v[:, (b,d)] where
    W (4096, 65) has two nonzeros per row ((1-t), t).  The t values are
    exact in bf16 for this index pattern, so v is split v = vhi + vlo
    (both bf16) and each output tile is computed with two accumulating
    bf16 matmuls into PSUM (1 cycle/row each vs 4 for fp32).
  - Output partition layout: partition jc holds j in [jc*32, jc*32+32) so
    each partition's free run (jf, d) is 16 KiB contiguous in HBM ->
    large efficient DMA descriptors.  4 passes x 4 batches, out staged in
    SBUF, DMA'd out in half-pass (4 MiB) chunks, double buffered.
"""

import sys

import numpy as np

if "/opt/trn_rl_repo" not in sys.path:
    sys.path.insert(0, "/opt/trn_rl_repo")

import ml_dtypes

import concourse.bass as bass
import concourse.mybir as mybir
import concourse.tile as tile
from concourse import bacc
from concourse.bass_utils import run_bass_kernel_spmd

BF16 = ml_dtypes.bfloat16

N_CORES = 8
B_FULL = 128
B_SHARD = 16  # batches per core
NK = 65  # keypoints
M = 4096  # output timesteps
D = 128  # feature dim
JC = 128  # coarse j (partition dim); j = jc*32 + jf
JF = 32  # fine j per partition
PASS_B = 4  # batches per pass
NPASS = B_SHARD // PASS_B
PN = PASS_B * D  # matmul moving free size (512)

_CACHE: dict = {}


# jf-chunk schedule per pass: tiny chunks first so the first output DMA
# fires early, then full-size (16-jf = 4 MiB) chunks once DMA saturates.
CHUNKS_P0 = [1, 1, 2, 3, 5, 8, 12]
CHUNKS_PN = [16, 16]
N_WARMUP_MM = 3  # dummy matmuls on junk data to ramp the PE pstate early
W_HEAD = 4  # jf columns of w included in the fast "head" input


def _build_program():
    nc = bacc.Bacc("TRN2", target_bir_lowering=False, debug=False)

    # head = [w cols for jf<W_SPLIT | vhi pass0 | vlo pass0]: the small
    # critical slice that gates the first output chunks, in ONE fast DMA.
    head_w = W_HEAD * JC + 2 * PN
    head = nc.dram_tensor("head", [NK, head_w], mybir.dt.bfloat16, kind="ExternalInput").ap()
    vhi = nc.dram_tensor("vhi", [NK, B_SHARD * D], mybir.dt.bfloat16, kind="ExternalInput").ap()
    vlo = nc.dram_tensor("vlo", [NK, B_SHARD * D], mybir.dt.bfloat16, kind="ExternalInput").ap()
    w = nc.dram_tensor("w", [NK, JF * JC], mybir.dt.bfloat16, kind="ExternalInput").ap()
    out = nc.dram_tensor("out", [B_SHARD, M, D], mybir.dt.float32, kind="ExternalOutput").ap()

    # HBM view: (jc, b, jf, d) so that per (jc, b) the (jf, d) run is 16KB
    # contiguous.
    out_r = out.rearrange("b (jc jf) d -> jc b jf d", jc=JC, jf=JF)

    with tile.TileContext(nc) as tc:
        with (
            tc.tile_pool(name="const", bufs=1) as const,
            tc.tile_pool(name="outp", bufs=4) as outp,
            tc.tile_pool(name="psum", bufs=8, space="PSUM") as psump,
        ):
            head_t = const.tile([NK, head_w], mybir.dt.bfloat16)
            w_t = const.tile([NK, JF * JC], mybir.dt.bfloat16)
            vhi_t = const.tile([NK, B_SHARD * D], mybir.dt.bfloat16)
            vlo_t = const.tile([NK, B_SHARD * D], mybir.dt.bfloat16)
            junk = const.tile([NK, D], mybir.dt.bfloat16)  # garbage is fine
            # Critical head slice first (one small DMA), then the bulk.
            nc.sync.dma_start(head_t[:], head)
            nc.sync.dma_start(w_t[:], w)
            nc.sync.dma_start(vhi_t[:], vhi)
            nc.sync.dma_start(vlo_t[:], vlo)

            # PE pstate/HAM warm-up: junk matmuls with no input dependencies.
            nc.gpsimd.memset(junk[:], 0.0)
            wps = psump.tile([JC, PN], mybir.dt.float32, tag="ps")
            for _ in range(N_WARMUP_MM):
                nc.tensor.matmul(wps[:, :D], junk[:, :JC], junk[:], start=True, stop=True)

            hd_hi = head_t[:, W_HEAD * JC:W_HEAD * JC + PN]
            hd_lo = head_t[:, W_HEAD * JC + PN:]
            for p in range(NPASS):
                rhi = hd_hi if p == 0 else vhi_t[:, p * PN:(p + 1) * PN]
                rlo = hd_lo if p == 0 else vlo_t[:, p * PN:(p + 1) * PN]
                chunks = CHUNKS_P0 if p == 0 else CHUNKS_PN
                jf0 = 0
                for ci, clen in enumerate(chunks):
                    ob = outp.tile([JC, PASS_B, clen, D], mybir.dt.float32, tag="ob")
                    for jfi in range(clen):
                        jf = jf0 + jfi
                        ps = psump.tile([JC, PN], mybir.dt.float32)
                        if p == 0 and jf < W_HEAD:
                            lhsT = head_t[:, jf * JC:(jf + 1) * JC]
                        else:
                            lhsT = w_t[:, jf * JC:(jf + 1) * JC]
                        nc.tensor.matmul(ps[:], lhsT, rhi, start=True, stop=False)
                        nc.tensor.matmul(ps[:], lhsT, rlo, start=False, stop=True)
                        dst = ob[:, :, jfi, :]
                        src = ps[:].rearrange("m (b d) -> m b d", b=PASS_B)
                        if jf % 2 == 0:
                            nc.vector.tensor_copy(dst, src)
                        else:
                            nc.scalar.copy(dst, src)
                    nc.sync.dma_start(
                        out_r[:, p * PASS_B:(p + 1) * PASS_B, jf0:jf0 + clen, :],
                        ob[:],
                    )
                    jf0 += clen
    return nc


def _get_program():
    if "nc" not in _CACHE:
        nc = _build_program()
        nc.compile()
        _CACHE["nc"] = nc
    return _CACHE["nc"]


def _make_weights(index: np.ndarray) -> np.ndarray:
    idx = np.asarray(index, dtype=np.int64)
    assert idx.shape == (NK,)
    xp = np.arange(idx[0] + 1, idx[-1] + 1)
    assert xp.shape == (M,)
    seg = np.searchsorted(idx, xp, side="left") - 1
    t = (xp - idx[seg]).astype(np.float32) / (idx[seg + 1] - idx[seg]).astype(np.float32)
    wmat = np.zeros((M, NK), dtype=np.float32)
    ar = np.arange(M)
    wmat[ar, seg] = 1.0 - t
    wmat[ar, seg + 1] = t
    # [j, k] -> [k, jf*128 + jc] with j = jc*32 + jf
    wk = wmat.reshape(JC, JF, NK).transpose(2, 1, 0).reshape(NK, JF * JC)
    return np.ascontiguousarray(wk).astype(BF16)


def kernel(index: np.ndarray, value: np.ndarray, _trace: bool = False):
    value = np.asarray(value, dtype=np.float32)
    assert value.shape == (B_FULL, NK, D)
    w_bf = _make_weights(index)

    vt = value.transpose(1, 0, 2)  # (k, b, d)
    in_maps = []
    for c in range(N_CORES):
        vc = np.ascontiguousarray(vt[:, c * B_SHARD:(c + 1) * B_SHARD, :]).reshape(NK, B_SHARD * D)
        vhi = vc.astype(BF16)
        vlo = (vc - vhi.astype(np.float32)).astype(BF16)
        head = np.ascontiguousarray(
            np.concatenate([w_bf[:, :W_HEAD * JC], vhi[:, :PN], vlo[:, :PN]], axis=1)
        )
        in_maps.append({"head": head, "vhi": vhi, "vlo": vlo, "w": w_bf})

    nc = _get_program()
    res = run_bass_kernel_spmd(nc, in_maps, core_ids=list(range(N_CORES)), trace=_trace)
    kernel.last_results = res
    out = np.concatenate([res.results[c]["out"] for c in range(N_CORES)], axis=0)
    return out


kernel.last_results = None

